# revision 1
# baseline (speedup 1.0000x reference)
"""Block-local self-attention (BLOCK_SIZE=64) Trainium2 Bass kernel.

Full inputs in, full output out. Sharding: batch*heads = 48 planes, 6 planes
per core across 8 cores (pure data parallel, no collectives).

Streaming pipeline. The kernel is HBM-bound (~12.7 MB/core at ~350 GB/s
=> ~37 us floor), so the schedule keeps the DMA system saturated:

  - Inputs are issued in consumption order, split across the two DGE
    queues (gpsimd: qt, sync: kt, va/mask alternating), chunked per
    (pair, superblock) "unit"; compute chases arrivals with a 3-unit lead.
  - Output DMAs are woven into the same queues at a 2-tile lag.
  - Compute pipelines at TILE granularity ([128 x 1024] PSUM, 2 banks,
    8 blocks = 512 seq positions x 2 planes) with a 4-deep PSUM rotation,
    so the mm1 -> exp -> mm2 -> normalize dependency loop spans 4 tiles of
    slack and never paces the pipeline; iteration t emits mm1+exp for tile
    t and mm2+normalize+store for tile t-2 (mm1 first, so a ready mm2 is
    never stuck in the tensor queue behind a data-starved mm1).

Per tile, the block-diagonal pair layout (chunk g of 128 cols = one query
block: cols 0:64 plane0-copy, 64:128 plane1-copy; rows 0:64 plane0 keys,
64:128 plane1 keys):

  mm1: per block, 2 matmuls (N=64, one per plane) write the two diagonal
      quadrants of the chunk: S^T = K_blk^T-contract-d Q_blk.
  exp: 2 ACT calls per tile ([64 x 512] strided) write the diagonal
      quadrants of a persistent bf16 P^T tile whose off-diagonal quadrants
      were zeroed once at kernel start => P^T chunks are block-diagonal.
  mm2: ONE matmul per block (N=65): lhsT = block-diag P^T chunk (128-deep
      contraction), rhs = V_aug pair-stacked [128, 65] (V*mask | mask)
      => both planes' outputs + softmax denominators in one instruction.
  normalize: reciprocal of denominators, times query-mask, times out rows;
      stores [128, 8, 64] f16 (no denominator column => 1 KB contiguous
      DMA descriptors).
"""

import numpy as np
import ml_dtypes

BS, H, S, D = 4, 12, 4096, 64
NCORES = 8
PLANES = BS * H          # 48
PPC = PLANES // NCORES   # 6 planes per core
PAIRS = PPC // 2         # 3 plane-pairs per core
NBLK = S // 64           # 64 key/query blocks per plane
NSB = 4                  # superblocks (units) per pair
SHIFT = -20.0            # range shift; cancels in the softmax ratio

_compiled = {}


def _build_nc(ppc=PPC):
    import concourse.bass as bass  # noqa: F401
    import concourse.mybir as mybir
    import concourse.tile as tile
    from concourse import bacc

    f32 = mybir.dt.float32
    bf16 = mybir.dt.bfloat16
    f16 = mybir.dt.float16
    EXP = mybir.ActivationFunctionType.Exp

    pairs = ppc // 2
    UNITS = pairs * NSB  # 12
    PRE = 3              # units of input lead

    nc = bacc.Bacc("TRN2", target_bir_lowering=False, debug=False)

    qt_d = nc.dram_tensor("qt", [pairs, 128, S], f16, kind="ExternalInput")
    kt_d = nc.dram_tensor("kt", [pairs, 128, S], f16, kind="ExternalInput")
    va_d = nc.dram_tensor("vaug", [pairs, 128, NBLK, D + 1], bf16, kind="ExternalInput")
    mk_d = nc.dram_tensor("maskt", [pairs, 128, NBLK], f32, kind="ExternalInput")
    out_d = nc.dram_tensor("out", [pairs, 128, NBLK, D], f16, kind="ExternalOutput")

    with tile.TileContext(nc) as tc:
        with (
            tc.tile_pool(name="qk", bufs=1) as qk_pool,
            tc.tile_pool(name="vio", bufs=1) as vio_pool,
            tc.tile_pool(name="oio", bufs=1) as oio_pool,
            tc.tile_pool(name="ptp", bufs=1) as pt_pool,
            tc.tile_pool(name="sm", bufs=4) as sm_pool,
            tc.tile_pool(name="cst", bufs=1) as cst_pool,
            tc.tile_pool(name="psa", bufs=4, space="PSUM") as psa_pool,
        ):
            bias_u = cst_pool.tile([128, 1], f32, name="bias_u")
            nc.vector.memset(bias_u[:], SHIFT)

            # Persistent bf16 P^T tiles; off-diagonal quadrants are zeroed
            # once here and never rewritten, keeping every chunk
            # block-diagonal across all reuses.
            pt_t = []
            for i in range(4):
                t = pt_pool.tile([128, 1024], bf16, name=f"pt{i}", tag=f"pt{i}")
                nc.vector.memset(t[:], 0.0)
                pt_t.append(t)

            qt_t, kt_t, va_t, mk_t, out_t = {}, {}, {}, {}, {}
            for pp in range(pairs):
                qt_t[pp] = qk_pool.tile([128, S], f16, name=f"qt_t{pp}", tag=f"qt{pp}")
                kt_t[pp] = qk_pool.tile([128, S], f16, name=f"kt_t{pp}", tag=f"kt{pp}")
                va_t[pp] = vio_pool.tile(
                    [128, NBLK, D + 1], bf16, name=f"va_t{pp}", tag=f"va{pp}")
                mk_t[pp] = sm_pool.tile(
                    [128, NBLK], f32, name=f"mk_t{pp}", tag=f"mk{pp}", bufs=1)
                out_t[pp] = oio_pool.tile(
                    [128, NBLK, D], f16, name=f"out_t{pp}", tag=f"out{pp}")

            def issue_qtkt(u, first=False):
                pp, sb = divmod(u, NSB)
                c0 = sb * 1024
                if first:
                    # finer chunks so the first tile's matmuls start earliest
                    for h in range(4):
                        sl = slice(c0 + 256 * h, c0 + 256 * (h + 1))
                        nc.gpsimd.dma_start(qt_t[pp][:, sl], qt_d[pp, :, sl])
                        nc.sync.dma_start(kt_t[pp][:, sl], kt_d[pp, :, sl])
                else:
                    sl = slice(c0, c0 + 1024)
                    nc.gpsimd.dma_start(qt_t[pp][:, sl], qt_d[pp, :, sl])
                    nc.sync.dma_start(kt_t[pp][:, sl], kt_d[pp, :, sl])

            def issue_vamk(u):
                pp, sb = divmod(u, NSB)
                bsl = slice(sb * 16, sb * 16 + 16)
                e_va = nc.gpsimd if u % 2 == 0 else nc.sync
                e_va.dma_start(va_t[pp][:, bsl, :], va_d[pp, :, bsl, :])
                if sb == 0:
                    e_mk = nc.sync if u % 2 == 0 else nc.gpsimd
                    e_mk.dma_start(mk_t[pp][:], mk_d[pp])

            def issue_inputs(u, first=False):
                issue_qtkt(u, first)
                issue_vamk(u)

            ps_live = {}

            # Per-TILE pipeline: a tile = [128, 1024] PSUM (2 banks) covering 8
            # blocks (512 seq positions x 2 planes). 4-deep rotation means the
            # mm1->exp->mm2->normalize loop spans 4 tiles of slack and never
            # paces the pipeline; the cadence is set by engine busy time.
            def mm1_act(t):
                pp, ch = divmod(t, 8)
                ps = psa_pool.tile([128, 1024], f32, name="psT", tag="psT")
                ps_live[t] = ps
                cbase = ch * 512
                # all plane0 matmuls first: the first exp call only reads
                # rows 0:64, so it can start once half of mm1 has finished
                for g in range(8):
                    c = cbase + g * 64
                    o = g * 128
                    nc.tensor.matmul(
                        ps[0:64, o:o + 64],
                        kt_t[pp][0:64, c:c + 64],
                        qt_t[pp][0:64, c:c + 64],
                        start=True, stop=True)
                for g in range(8):
                    c = cbase + g * 64
                    o = g * 128
                    nc.tensor.matmul(
                        ps[64:128, o + 64:o + 128],
                        kt_t[pp][64:128, c:c + 64],
                        qt_t[pp][64:128, c:c + 64],
                        start=True, stop=True)
                pt = pt_t[t % 4]
                psv = ps[:].rearrange("p (g a b) -> p g a b", g=8, a=2, b=64)
                ptv = pt[:].rearrange("p (g a b) -> p g a b", g=8, a=2, b=64)
                nc.scalar.activation(
                    ptv[0:64, :, 0, :], psv[0:64, :, 0, :], EXP,
                    bias=bias_u[0:64, :])
                nc.scalar.activation(
                    ptv[64:128, :, 1, :], psv[64:128, :, 1, :], EXP,
                    bias=bias_u[64:128, :])

            def mm2_norm_out(t):
                pp, ch = divmod(t, 8)
                ps = ps_live.pop(t)
                pt = pt_t[t % 4]
                b0 = ch * 8
                for g in range(8):
                    o = g * 128
                    nc.tensor.matmul(
                        ps[0:128, o:o + 65],
                        pt[0:128, o:o + 128],
                        va_t[pp][:, b0 + g, :],
                        start=True, stop=True)
                psq = ps[:].rearrange("p (g x) -> p g x", g=8)
                rc = sm_pool.tile([128, 8], f32, name="rc", tag="rc")
                rs = sm_pool.tile([128, 8], f32, name="rs", tag="rs")
                nc.vector.reciprocal(rc[:], psq[:, :, 64])
                nc.vector.tensor_mul(rs[:], rc[:], mk_t[pp][:, b0:b0 + 8])
                outv = out_t[pp][:, b0:b0 + 8, :]
                rs_b = rs[:].unsqueeze(2).broadcast_to((128, 8, 64))
                nc.vector.tensor_mul(outv, psq[:, :, 0:64], rs_b)

            def out_dma(t, split=False):
                pp, ch = divmod(t, 8)
                b0 = ch * 8
                if split:
                    # last tiles: halve across both queues so the final
                    # transfers run in parallel and the drain starts sooner
                    nc.sync.dma_start(
                        out_d[pp, :, b0:b0 + 4, :], out_t[pp][:, b0:b0 + 4, :])
                    nc.gpsimd.dma_start(
                        out_d[pp, :, b0 + 4:b0 + 8, :],
                        out_t[pp][:, b0 + 4:b0 + 8, :])
                else:
                    e = nc.sync if t % 2 == 0 else nc.gpsimd
                    e.dma_start(
                        out_d[pp, :, b0:b0 + 8, :], out_t[pp][:, b0:b0 + 8, :])

            for u in range(PRE):
                issue_inputs(u, first=(u == 0))
            NT = UNITS * 2  # 24 tiles
            for t in range(NT + 2):
                if t % 2 == 0 and t // 2 + PRE < UNITS:
                    issue_inputs(t // 2 + PRE)
                if t < NT:
                    mm1_act(t)
                if 0 <= t - 2 < NT:
                    mm2_norm_out(t - 2)
                    out_dma(t - 2, split=(t - 2 >= NT - 2))

    nc.compile()
    return nc


def _get_nc(ppc=PPC):
    if ppc not in _compiled:
        _compiled[ppc] = _build_nc(ppc)
    return _compiled[ppc]


def _pack(Q, K, V, mask):
    Qp = np.asarray(Q, np.float32).reshape(PLANES, S, D)
    Kp = np.asarray(K, np.float32).reshape(PLANES, S, D)
    Vp = np.asarray(V, np.float32).reshape(PLANES, S, D)
    maskp = np.asarray(mask, np.float32)[np.repeat(np.arange(BS), H)]  # [48, S]

    # [ncores, pairs, 128, S]: rows 0:64 even plane's d, 64:128 odd plane's d
    qt = np.ascontiguousarray(Qp.transpose(0, 2, 1)).astype(np.float16).reshape(
        NCORES, PAIRS, 128, S)
    kt = np.ascontiguousarray(Kp.transpose(0, 2, 1)).astype(np.float16).reshape(
        NCORES, PAIRS, 128, S)

    # V_aug pair-stacked per block: [pair, r(128), blk, c]; rows 0:64 even
    # plane (seq = 64*blk + r), rows 64:128 odd plane; c 0:64 = V*mask,
    # c 64 = mask (softmax denominator via the mm2 ones-column trick).
    va = np.empty((PLANES, S, D + 1), np.float32)
    va[:, :, :D] = Vp * maskp[:, :, None]
    va[:, :, D] = maskp
    va = va.reshape(PLANES, NBLK, 64, D + 1).transpose(0, 2, 1, 3)  # [pl, r, blk, c]
    va = va.reshape(PLANES // 2, 2 * 64, NBLK, D + 1)  # pair-stack rows
    va = np.ascontiguousarray(va).astype(ml_dtypes.bfloat16).reshape(
        NCORES, PAIRS, 128, NBLK, D + 1)

    # query mask, same pair-stacked [pair, r, blk] layout
    mt = maskp.reshape(PLANES, NBLK, 64).transpose(0, 2, 1)  # [pl, r, blk]
    mk = np.ascontiguousarray(mt.reshape(PLANES // 2, 128, NBLK)).reshape(
        NCORES, PAIRS, 128, NBLK)

    return [
        {"qt": qt[c], "kt": kt[c], "vaug": va[c], "maskt": mk[c]}
        for c in range(NCORES)
    ]


def _unpack(results):
    # results[c]["out"]: [PAIRS, 128, blk, d]; row r: plane = 2pp + (r>=64),
    # seq = 64*blk + (r % 64)
    full = np.concatenate(
        [results[c]["out"] for c in range(NCORES)], axis=0).astype(np.float32)
    full = full.reshape(PLANES // 2, 2, 64, NBLK, D).transpose(0, 1, 3, 2, 4)
    return np.ascontiguousarray(full).reshape(BS, H, S, D)


def run_hw(inputs, trace=False):
    from concourse.bass_utils import run_bass_kernel_spmd

    nc = _get_nc()
    in_maps = _pack(inputs["Q"], inputs["K"], inputs["V"], inputs["mask"])
    res = run_bass_kernel_spmd(nc, in_maps, list(range(NCORES)), trace=trace)
    return _unpack(res.results), res


def kernel(Q, K, V, mask):
    out, _ = run_hw({"Q": Q, "K": K, "V": V, "mask": mask}, trace=False)
    return out



# revision 5
# speedup vs baseline: 1.1571x; 1.1571x over previous
"""Block-local self-attention (BLOCK_SIZE=64) Trainium2 Bass kernel, v2.

Full inputs in, full output out. Sharding: batch*heads = 48 planes, 6 planes
per core across 8 cores (pure data parallel, no collectives).

HBM floor is ~12.6 MB/core (Q,K f16 + V_aug bf16 + out f16) ~= 35 us at
358 GB/s; the schedule aims to hide all compute under that DMA curve.

Per-core layout: planes are processed in PAIRS stacked on SBUF partitions
(rows 0:64 = even plane, 64:128 = odd plane). A TILE is 8 query blocks of
one pair = [128, 512] of scores.

Compute per tile (all pair-concurrent via PE array quadrants):
  mm1: per block g, two 64x64x64 matmuls — plane0 in array quadrant
      (rows 0:64, cols 0:64) -> ps1[0:64, g*64:+64], plane1 in quadrant
      (64:128, 64:128) -> ps1[64:128, g*64:+64]. Interleaved issue so the
      two quadrant chains execute concurrently. ps1 = [128,512] = 1 bank,
      DENSE (both planes share columns).
  exp: ONE activation call [128, 512] (contiguous) ps1 -> pt (bf16, SBUF),
      bias = SHIFT (cancels in the softmax ratio).
  mm2: per block g, two concurrent 64-deep matmuls: plane0
      lhsT = pt[0:64, g*64:+64] (keys x queries), rhs = va[0:64, blk, 0:65]
      (V*mask | mask) -> ps2[0:64, g*128 : g*128+65]; plane1 same in the
      opposite quadrant. Column 64 = softmax denominator (ones-column
      trick). ps2 blocks at stride 128 so no 65-col window crosses a PSUM
      bank boundary.
  normalize: reciprocal of denominators [128,8], broadcast-multiply onto
      [128, 8, 64], store f16. (Query-side mask is applied on the HOST
      after unpacking — removes mask DMA + one vector op per tile.)

DMA: three rings so input flow never blocks behind compute-dependent
stores: sync (HWDGE) = Q^T,K^T; scalar (HWDGE) = V_aug (no deps, never
stalls EXP); gpsimd (SWDGE) = output stores (waits only block later
stores). Inputs chunked per (pair, superblock=2 tiles); first unit split
in half so tile 0's deps are minimal. A warmup EXP at t=0 pulls the
~2.7us ACT table load into the DMA ramp.
"""

import numpy as np
import ml_dtypes

BS, H, S, D = 4, 12, 4096, 64
NCORES = 8
PLANES = BS * H          # 48
PPC = PLANES // NCORES   # 6 planes per core
PAIRS = PPC // 2         # 3 plane-pairs per core
NBLK = S // 64           # 64 key/query blocks per plane
NSB = 4                  # superblocks (units) per pair
SHIFT = -20.0            # range shift; cancels in the softmax ratio

_compiled = {}


def _build_nc(ppc=PPC):
    import concourse.bass as bass  # noqa: F401
    import concourse.mybir as mybir
    import concourse.tile as tile
    from concourse import bacc

    f32 = mybir.dt.float32
    bf16 = mybir.dt.bfloat16
    f16 = mybir.dt.float16
    EXP = mybir.ActivationFunctionType.Exp

    pairs = ppc // 2
    UNITS = pairs * NSB  # 12
    PRE = 3              # units of input lead

    nc = bacc.Bacc("TRN2", target_bir_lowering=False, debug=False)

    qt_d = nc.dram_tensor("qt", [pairs, 128, S], f16, kind="ExternalInput")
    kt_d = nc.dram_tensor("kt", [pairs, 128, S], f16, kind="ExternalInput")
    va_d = nc.dram_tensor("vaug", [pairs, 128, NBLK, D + 1], bf16, kind="ExternalInput")
    out_d = nc.dram_tensor("out", [pairs, 128, NBLK, D], f16, kind="ExternalOutput")

    with tile.TileContext(nc) as tc:
        with (
            tc.tile_pool(name="qk", bufs=1) as qk_pool,
            tc.tile_pool(name="vio", bufs=1) as vio_pool,
            tc.tile_pool(name="oio", bufs=1) as oio_pool,
            tc.tile_pool(name="ptp", bufs=3) as pt_pool,
            tc.tile_pool(name="sm", bufs=4) as sm_pool,
            tc.tile_pool(name="ps1", bufs=3, space="PSUM") as ps1_pool,
            tc.tile_pool(name="ps2", bufs=2, space="PSUM") as ps2_pool,
        ):
            # warmup: trigger the ACT exp-table load during the DMA ramp
            bias_u = sm_pool.tile([128, 1], f32, name="bias_u", tag="bias_u", bufs=1)
            nc.vector.memset(bias_u[:], SHIFT)
            # warmup: trigger the ACT exp-table load during the DMA ramp
            wu = sm_pool.tile([128, 1], f32, name="wu", tag="wu", bufs=1)
            nc.scalar.activation(wu[:], bias_u[:], EXP, bias=bias_u[:])

            qt_t, kt_t, va_t, out_t = {}, {}, {}, {}
            for pp in range(pairs):
                qt_t[pp] = qk_pool.tile([128, S], f16, name=f"qt_t{pp}", tag=f"qt{pp}")
                kt_t[pp] = qk_pool.tile([128, S], f16, name=f"kt_t{pp}", tag=f"kt{pp}")
                va_t[pp] = vio_pool.tile(
                    [128, NBLK, D + 1], bf16, name=f"va_t{pp}", tag=f"va{pp}")
                out_t[pp] = oio_pool.tile(
                    [128, NBLK, D], f16, name=f"out_t{pp}", tag=f"out{pp}")

            def issue_inputs(u, first=False):
                pp, sb = divmod(u, NSB)
                c0 = sb * 1024
                if first:
                    # split so tile 0's q/k dep is just its own 512 cols
                    for h in range(2):
                        sl = slice(c0 + 512 * h, c0 + 512 * (h + 1))
                        nc.sync.dma_start(qt_t[pp][:, sl], qt_d[pp, :, sl])
                        nc.sync.dma_start(kt_t[pp][:, sl], kt_d[pp, :, sl])
                else:
                    sl = slice(c0, c0 + 1024)
                    nc.sync.dma_start(qt_t[pp][:, sl], qt_d[pp, :, sl])
                    nc.sync.dma_start(kt_t[pp][:, sl], kt_d[pp, :, sl])
                bsl = slice(sb * 16, sb * 16 + 16)
                nc.scalar.dma_start(va_t[pp][:, bsl, :], va_d[pp, :, bsl, :])

            ps1_live = {}
            pt_live = {}

            def mm1_exp(t):
                pp, ch = divmod(t, 8)
                ps = ps1_pool.tile([128, 512], f32, name="ps1", tag="ps1")
                ps1_live[t] = ps
                cbase = ch * 512
                # interleave the two quadrant chains so they run concurrently
                for g in range(8):
                    c = cbase + g * 64
                    o = g * 64
                    nc.tensor.matmul(
                        ps[0:64, o:o + 64],
                        kt_t[pp][0:64, c:c + 64],
                        qt_t[pp][0:64, c:c + 64],
                        start=True, stop=True)
                    nc.tensor.matmul(
                        ps[64:128, o:o + 64],
                        kt_t[pp][64:128, c:c + 64],
                        qt_t[pp][64:128, c:c + 64],
                        start=True, stop=True)
                pt = pt_pool.tile([128, 512], bf16, name="pt", tag="pt")
                pt_live[t] = pt
                nc.scalar.activation(pt[:], ps[:], EXP, bias=bias_u[:])

            def mm2_norm(t):
                pp, ch = divmod(t, 8)
                ps1_live.pop(t)
                pt = pt_live.pop(t)
                ps = ps2_pool.tile([128, 8, 128], f32, name="ps2", tag="ps2")
                b0 = ch * 8
                for g in range(8):
                    o = g * 64
                    nc.tensor.matmul(
                        ps[0:64, g, 0:65],
                        pt[0:64, o:o + 64],
                        va_t[pp][0:64, b0 + g, :],
                        start=True, stop=True)
                    nc.tensor.matmul(
                        ps[64:128, g, 0:65],
                        pt[64:128, o:o + 64],
                        va_t[pp][64:128, b0 + g, :],
                        start=True, stop=True)
                rc = sm_pool.tile([128, 8], f32, name="rc", tag="rc")
                nc.vector.reciprocal(rc[:], ps[:, :, 64])
                outv = out_t[pp][:, b0:b0 + 8, :]
                rc_b = rc[:].unsqueeze(2).broadcast_to((128, 8, 64))
                nc.vector.tensor_mul(outv, ps[:, :, 0:64], rc_b)

            def out_dma(s):
                pp, sb = divmod(s, NSB)
                b0 = sb * 16
                nc.gpsimd.dma_start(
                    out_d[pp, :, b0:b0 + 16, :], out_t[pp][:, b0:b0 + 16, :])

            for u in range(PRE):
                issue_inputs(u, first=(u == 0))
            NT = UNITS * 2  # 24 tiles
            for t in range(NT + 2):
                if t % 2 == 0 and t // 2 + PRE < UNITS:
                    issue_inputs(t // 2 + PRE)
                if 0 <= t - 2 < NT:
                    mm2_norm(t - 2)
                if t < NT:
                    mm1_exp(t)
                # superblock s tiles (2s, 2s+1) normalized by iteration 2s+3
                if t >= 4 and (t - 4) % 2 == 0 and (t - 4) // 2 < UNITS:
                    out_dma((t - 4) // 2)
            out_dma(UNITS - 1)

    nc.compile()
    return nc


def _get_nc(ppc=PPC):
    if ppc not in _compiled:
        _compiled[ppc] = _build_nc(ppc)
    return _compiled[ppc]


def _pack(Q, K, V, mask):
    Qp = np.asarray(Q, np.float32).reshape(PLANES, S, D)
    Kp = np.asarray(K, np.float32).reshape(PLANES, S, D)
    Vp = np.asarray(V, np.float32).reshape(PLANES, S, D)
    maskp = np.asarray(mask, np.float32)[np.repeat(np.arange(BS), H)]  # [48, S]

    # [ncores, pairs, 128, S]: rows 0:64 even plane's d, 64:128 odd plane's d
    qt = np.ascontiguousarray(Qp.transpose(0, 2, 1)).astype(np.float16).reshape(
        NCORES, PAIRS, 128, S)
    kt = np.ascontiguousarray(Kp.transpose(0, 2, 1)).astype(np.float16).reshape(
        NCORES, PAIRS, 128, S)

    # V_aug pair-stacked per block: [pair, r(128), blk, c]; rows 0:64 even
    # plane (seq = 64*blk + r), rows 64:128 odd plane; c 0:64 = V*mask,
    # c 64 = mask (softmax denominator via the mm2 ones-column trick).
    va = np.empty((PLANES, S, D + 1), np.float32)
    va[:, :, :D] = Vp * maskp[:, :, None]
    va[:, :, D] = maskp
    va = va.reshape(PLANES, NBLK, 64, D + 1).transpose(0, 2, 1, 3)  # [pl, r, blk, c]
    va = va.reshape(PLANES // 2, 2 * 64, NBLK, D + 1)  # pair-stack rows
    va = np.ascontiguousarray(va).astype(ml_dtypes.bfloat16).reshape(
        NCORES, PAIRS, 128, NBLK, D + 1)

    return [
        {"qt": qt[c], "kt": kt[c], "vaug": va[c]}
        for c in range(NCORES)
    ]


def _unpack(results, mask):
    # results[c]["out"]: [PAIRS, 128, blk, d]; row r: plane = 2pp + (r>=64),
    # seq = 64*blk + (r % 64)
    full = np.concatenate(
        [results[c]["out"] for c in range(NCORES)], axis=0).astype(np.float32)
    full = full.reshape(PLANES // 2, 2, 64, NBLK, D).transpose(0, 1, 3, 2, 4)
    out = np.ascontiguousarray(full).reshape(BS, H, S, D)
    # query-side mask: zero rows whose query position is masked
    out *= np.asarray(mask, np.float32)[:, None, :, None]
    return out


def run_hw(inputs, trace=False):
    from concourse.bass_utils import run_bass_kernel_spmd

    nc = _get_nc()
    in_maps = _pack(inputs["Q"], inputs["K"], inputs["V"], inputs["mask"])
    res = run_bass_kernel_spmd(nc, in_maps, list(range(NCORES)), trace=trace)
    return _unpack(res.results, inputs["mask"]), res


def kernel(Q, K, V, mask):
    out, _ = run_hw({"Q": Q, "K": K, "V": V, "mask": mask}, trace=False)
    return out
